# revision 11
# baseline (speedup 1.0000x reference)
"""Trainium2 Bass kernel: single-head attention with query-axis softmax.

Reference semantics (per batch element b):
    q = X @ Wq.T + bq ; k = X @ Wk.T + bk ; v = X @ Wv.T + bv          # [T,E]
    s = q @ k.T / sqrt(E), s[i,j] = -inf for j > i (strict upper tri)
    attn = softmax(s, axis=-2)          # over the QUERY axis i (faithful)
    out = attn @ v                      # [T,E]

Sharding: data-parallel over batch. B=8 batch elements -> one NeuronCore
each; host transposes/casts inputs, device computes, host stacks outputs.

Device strategy (per core): compute qT,kT in feature-major layout [E,T] and
v in token-major [T,E]. Scores are built transposed, sT[j,i] = k_j . q_i, so
the softmax reduction (over i) runs along the free axis. exp rows are
normalized implicitly by folding 1/colsum[j] into v[j,:]. The causal
structure (only i >= j is live) halves the score and attn@v matmul work.
All matmul operands are bf16 (fp32 accumulation in PSUM).
"""

import math
from contextlib import ExitStack

import ml_dtypes
import numpy as np

import concourse.bacc as bacc
import concourse.bass as bass
import concourse.tile as tile
from concourse import mybir
from concourse.bass_utils import run_bass_kernel_spmd

BF16 = ml_dtypes.bfloat16

P = 128          # partitions
T = 2048         # tokens
E = 1024         # embed
B = 8            # batch == n_cores
EO = E // P      # 8 contraction tiles
FO = E // P      # 8 feature tiles
NT = T // P      # 16 token tiles
TCH = T // 512   # 4 chunks of 512 tokens
SCALE = 1.0 / math.sqrt(E)   # 1/32
NEG = -1.0e30

_CACHE = {}


def _build_nc():
    f32 = mybir.dt.float32
    bf16 = mybir.dt.bfloat16
    Act = mybir.ActivationFunctionType

    nc = bacc.Bacc(None, target_bir_lowering=False)

    # [p, eo, t] with X^T[e, t] = X[t, e], e = eo*128 + p
    XT = nc.dram_tensor("XT", [P, EO, T], bf16, kind="ExternalInput")
    # [fo, p, eo, c] = W.T[eo*128+p, fo*128+c]  (stationary operand blocks)
    WQ = nc.dram_tensor("WQ", [FO, P, EO, P], bf16, kind="ExternalInput")
    WK = nc.dram_tensor("WK", [FO, P, EO, P], bf16, kind="ExternalInput")
    # [p, eo, f] = Wv.T[eo*128+p, f]  (moving operand, f contiguous)
    WV = nc.dram_tensor("WV", [P, EO, E], bf16, kind="ExternalInput")
    BQ = nc.dram_tensor("BQ", [P, FO], f32, kind="ExternalInput")
    BK = nc.dram_tensor("BK", [P, FO], f32, kind="ExternalInput")
    BV = nc.dram_tensor("BV", [P, E], bf16, kind="ExternalInput")   # bv row bcast
    MSK = nc.dram_tensor("MSK", [P, P], f32, kind="ExternalInput")  # 0 / -1e30
    OUT = nc.dram_tensor("OUT", [T, E], f32, kind="ExternalOutput")

    with tile.TileContext(nc) as tc, ExitStack() as ctx:
        persist = ctx.enter_context(tc.tile_pool(name="persist", bufs=1))
        wpool = ctx.enter_context(tc.tile_pool(name="wpool", bufs=2))
        small = ctx.enter_context(tc.tile_pool(name="small", bufs=1))
        outst = ctx.enter_context(tc.tile_pool(name="outst", bufs=2))
        ps = ctx.enter_context(tc.tile_pool(name="ps", bufs=4, space="PSUM"))
        ps3 = ctx.enter_context(tc.tile_pool(name="ps3", bufs=4, space="PSUM"))

        xt = persist.tile([P, EO, T], bf16)    # 32 KB/part
        qt = persist.tile([P, FO, T], bf16)    # 32
        kt = persist.tile([P, FO, T], bf16)    # 32
        v = persist.tile([P, NT, E], bf16)     # 32
        wv = persist.tile([P, EO, E], bf16)    # 16
        # unnormalized exp(scores^T) rows for each key tile, exact widths (~34)
        et = [
            persist.tile([P, T - jt * P], bf16, tag=f"et{jt}", name=f"et{jt}")
            for jt in range(NT)
        ]
        sums = persist.tile([P, NT, TCH], f32)
        rcol = persist.tile([P, NT, 1], f32)

        bq_sb = small.tile([P, FO], f32)
        bk_sb = small.tile([P, FO], f32)
        bv_sb = small.tile([P, E], bf16)
        msk_sb = small.tile([P, P], f32)
        ones_sb = small.tile([P, P], bf16)

        nc.sync.dma_start(xt[:], XT[:])
        nc.sync.dma_start(wv[:], WV[:])
        nc.sync.dma_start(bq_sb[:], BQ[:])
        nc.sync.dma_start(bk_sb[:], BK[:])
        nc.sync.dma_start(bv_sb[:], BV[:])
        nc.sync.dma_start(msk_sb[:], MSK[:])
        nc.vector.memset(ones_sb[:], 1.0 / P)

        # ---- Phase 1a: qT, kT projections (feature-major) -------------------
        # qT[f, t] = sum_e Wq.T[e, f] * X.T[e, t]   (+ bq[f] per-partition)
        for W, bsb, dst in ((WQ, bq_sb, qt), (WK, bk_sb, kt)):
            for fo in range(FO):
                wti = wpool.tile([P, EO, P], bf16, tag="w")
                nc.sync.dma_start(wti[:], W[fo])
                for tci in range(TCH):
                    pt = ps.tile([P, 512], mybir.dt.float32, tag="ps")
                    for eo in range(EO):
                        nc.tensor.matmul(
                            pt[:],
                            wti[:, eo, :],
                            xt[:, eo, tci * 512:(tci + 1) * 512],
                            start=(eo == 0),
                            stop=(eo == EO - 1),
                        )
                    # psum + per-partition bias -> bf16 SBUF (ScalarE)
                    nc.scalar.activation(
                        out=dst[:, fo, tci * 512:(tci + 1) * 512],
                        in_=pt[:],
                        func=Act.Identity,
                        bias=bsb[:, fo:fo + 1],
                    )

        # ---- Phase 1b: v projection (token-major) ---------------------------
        # v[t, f] = sum_e X.T[e, t] * Wv.T[e, f] + bv[f]
        for to in range(NT):
            for half in range(2):
                pt = ps.tile([P, 512], mybir.dt.float32, tag="ps")
                for eo in range(EO):
                    nc.tensor.matmul(
                        pt[:],
                        xt[:, eo, to * P:(to + 1) * P],
                        wv[:, eo, half * 512:(half + 1) * 512],
                        start=(eo == 0),
                        stop=False,
                    )
                # bias via rank-128 matmul: sum_k (1/128) * bv[f]
                nc.tensor.matmul(
                    pt[:],
                    ones_sb[:],
                    bv_sb[:, half * 512:(half + 1) * 512],
                    start=False,
                    stop=True,
                )
                nc.scalar.copy(
                    out=v[:, to, half * 512:(half + 1) * 512], in_=pt[:]
                )

        # ---- Phase 2+3 interleaved over key/query tiles ---------------------
        for jt in range(NT):
            n_i = T - jt * P          # live columns i >= jt*128
            nch = (n_i + 511) // 512
            for c in range(nch):
                i0 = jt * P + c * 512
                w = min(512, T - i0)
                pt = ps.tile([P, 512], mybir.dt.float32, tag="ps")
                for fo in range(FO):
                    nc.tensor.matmul(
                        pt[:, :w],
                        kt[:, fo, jt * P:(jt + 1) * P],
                        qt[:, fo, i0:i0 + w],
                        start=(fo == 0),
                        stop=(fo == FO - 1),
                    )
                if c == 0:
                    # causal mask on the diagonal 128x128 block (additive -1e30)
                    nc.vector.tensor_add(
                        out=pt[:, :P], in0=pt[:, :P], in1=msk_sb[:]
                    )
                nc.scalar.activation(
                    out=et[jt][:, c * 512:c * 512 + w],
                    in_=pt[:, :w],
                    func=Act.Exp,
                    scale=SCALE,
                    accum_out=sums[:, jt, c:c + 1],
                )
            # softmax denominator for this key tile; fold 1/colsum into v
            nc.vector.tensor_reduce(
                out=rcol[:, jt, :],
                in_=sums[:, jt, :nch],
                axis=mybir.AxisListType.X,
                op=mybir.AluOpType.add,
            )
            nc.vector.reciprocal(out=rcol[:, jt, :], in_=rcol[:, jt, :])
            nc.vector.tensor_scalar_mul(
                out=v[:, jt, :], in0=v[:, jt, :], scalar1=rcol[:, jt, :]
            )

            # Phase 3: out rows for query tile it == jt (needs et[0..jt], v'[0..jt])
            it = jt
            po0 = ps3.tile([P, 512], mybir.dt.float32, tag="ps3")
            po1 = ps3.tile([P, 512], mybir.dt.float32, tag="ps3")
            for j2 in range(it + 1):
                off = (it - j2) * P
                lhs = et[j2][:, off:off + P]
                nc.tensor.matmul(
                    po0[:], lhs, v[:, j2, 0:512],
                    start=(j2 == 0), stop=(j2 == it),
                )
                nc.tensor.matmul(
                    po1[:], lhs, v[:, j2, 512:1024],
                    start=(j2 == 0), stop=(j2 == it),
                )
            ob = outst.tile([P, E], mybir.dt.float32, tag="ob")
            nc.scalar.copy(out=ob[:, 0:512], in_=po0[:])
            nc.vector.tensor_copy(out=ob[:, 512:1024], in_=po1[:])
            nc.sync.dma_start(OUT[it * P:(it + 1) * P, :], ob[:])

    nc.compile()
    return nc


def _prep_inputs(X, Wq, bq, Wk, bk, Wv, bv):
    """Host-side reshape/cast into the layouts the device program expects."""
    X = np.asarray(X, dtype=np.float32)

    # XT[b, p, eo, t] = X[b, t, eo*128+p]
    xt4 = np.ascontiguousarray(
        X.transpose(0, 2, 1).reshape(B, EO, P, T).transpose(0, 2, 1, 3)
    ).astype(BF16)

    def wqk(Wm):
        # [fo, p, eo, c] = W.T[eo*128+p, fo*128+c]
        WmT = np.asarray(Wm, dtype=np.float32).T
        return np.ascontiguousarray(
            WmT.reshape(EO, P, FO, P).transpose(2, 1, 0, 3)
        ).astype(BF16)

    wq4 = wqk(Wq)
    wk4 = wqk(Wk)
    # WV: [p, eo, f] = Wv.T[eo*128+p, f]
    WvT = np.asarray(Wv, dtype=np.float32).T
    wv4 = np.ascontiguousarray(
        WvT.reshape(EO, P, E).transpose(1, 0, 2)
    ).astype(BF16)

    def b2(bm):
        return np.ascontiguousarray(
            np.asarray(bm, dtype=np.float32).reshape(FO, P).T
        )

    bq2 = b2(bq)
    bk2 = b2(bk)
    bvr = np.ascontiguousarray(
        np.broadcast_to(np.asarray(bv, dtype=np.float32), (P, E))
    ).astype(BF16)

    ii = np.arange(P)
    mask = np.where(ii[None, :] >= ii[:, None], 0.0, NEG).astype(np.float32)

    shared = {"WQ": wq4, "WK": wk4, "WV": wv4, "BQ": bq2, "BK": bk2,
              "BV": bvr, "MSK": mask}
    return [dict(shared, XT=np.ascontiguousarray(xt4[b])) for b in range(B)]


def run_sharded(inputs, trace=False, **kwargs):
    """Build (cached), run on 8 cores, gather. Returns (out, BassKernelResults)."""
    if "nc" not in _CACHE:
        _CACHE["nc"] = _build_nc()
    nc = _CACHE["nc"]
    in_maps = _prep_inputs(**inputs)
    res = run_bass_kernel_spmd(
        nc, in_maps, core_ids=list(range(B)), trace=trace, **kwargs
    )
    out = np.stack([np.asarray(r["OUT"], dtype=np.float32) for r in res.results])
    return out, res


def kernel(**inputs) -> np.ndarray:
    out, _ = run_sharded(inputs)
    return out


# revision 29
# speedup vs baseline: 1.0790x; 1.0790x over previous
"""Trainium2 Bass kernel: single-head attention with query-axis softmax.

Reference semantics (per batch element b):
    q = X @ Wq.T + bq ; k = X @ Wk.T + bk ; v = X @ Wv.T + bv          # [T,E]
    s = q @ k.T / sqrt(E), s[i,j] = -inf for j > i (strict upper tri)
    attn = softmax(s, axis=-2)          # over the QUERY axis i (faithful)
    out = attn @ v                      # [T,E]

Sharding: data-parallel over batch. B=8 batch elements -> one NeuronCore
each; host transposes/casts inputs, device computes, host stacks outputs.

Device strategy (per core): compute qT,kT in feature-major layout [E,T] and
v in token-major [T,E]. Scores are built transposed, sT[j,i] = k_j . q_i, so
the softmax reduction (over i) runs along the free axis. exp rows are
normalized implicitly by folding 1/colsum[j] into v[j,:]. The causal
structure (only i >= j is live) halves the score and attn@v matmul work.
All matmul operands are bf16 (fp32 accumulation in PSUM).
"""

import math
from contextlib import ExitStack

import ml_dtypes
import numpy as np

import concourse.bacc as bacc
import concourse.bass as bass
import concourse.tile as tile
from concourse import mybir
from concourse.bass_utils import run_bass_kernel_spmd
from concourse.tile import add_dep_helper

BF16 = ml_dtypes.bfloat16

P = 128          # partitions
T = 2048         # tokens
E = 1024         # embed
B = 8            # batch == n_cores
EO = E // P      # 8 contraction tiles
FO = E // P      # 8 feature tiles
NT = T // P      # 16 token tiles
TCH = T // 512   # 4 chunks of 512 tokens
SCALE = 1.0 / math.sqrt(E)   # 1/32
NEG = -1.0e30

_CACHE = {}


def _build_nc():
    f32 = mybir.dt.float32
    bf16 = mybir.dt.bfloat16
    Act = mybir.ActivationFunctionType

    nc = bacc.Bacc(None, target_bir_lowering=False)

    # [p, eo, t] with X^T[e, t] = X[t, e], e = eo*128 + p
    XT = nc.dram_tensor("XT", [P, EO, T], bf16, kind="ExternalInput")
    # [fo, p, eo, c] = W.T[eo*128+p, fo*128+c]  (stationary operand blocks)
    WQ = nc.dram_tensor("WQ", [FO, P, EO, P], bf16, kind="ExternalInput")
    WK = nc.dram_tensor("WK", [FO, P, EO, P], bf16, kind="ExternalInput")
    # [p, eo, f] = Wv.T[eo*128+p, f]  (moving operand, f contiguous)
    WV = nc.dram_tensor("WV", [P, EO, E], bf16, kind="ExternalInput")
    BQ = nc.dram_tensor("BQ", [P, FO], f32, kind="ExternalInput")
    BK = nc.dram_tensor("BK", [P, FO], f32, kind="ExternalInput")
    BV = nc.dram_tensor("BV", [P, E], bf16, kind="ExternalInput")   # bv row bcast
    MSK = nc.dram_tensor("MSK", [P, P], f32, kind="ExternalInput")  # 0 / -1e30
    OUT = nc.dram_tensor("OUT", [T, E], f32, kind="ExternalOutput")

    with tile.TileContext(nc) as tc, ExitStack() as ctx:
        persist = ctx.enter_context(tc.tile_pool(name="persist", bufs=1))
        wpool = ctx.enter_context(tc.tile_pool(name="wpool", bufs=2))
        small = ctx.enter_context(tc.tile_pool(name="small", bufs=1))
        outst = ctx.enter_context(tc.tile_pool(name="outst", bufs=2))
        ps = ctx.enter_context(tc.tile_pool(name="ps", bufs=4, space="PSUM"))
        ps3 = ctx.enter_context(tc.tile_pool(name="ps3", bufs=4, space="PSUM"))

        # four 512-token chunk tiles so phase-1 can start before X is resident
        xtc = [
            persist.tile([P, EO, 512], bf16, tag=f"xt{i}", name=f"xt{i}")
            for i in range(TCH)
        ]                                      # 32 KB/part total
        qt = persist.tile([P, FO, T], bf16)    # 32
        kt = persist.tile([P, FO, T], bf16)    # 32
        v = persist.tile([P, NT, E], bf16)     # 32
        wv = persist.tile([P, EO, E], bf16)    # 16
        # unnormalized exp(scores^T) rows for each key tile, exact widths (~34)
        et = [
            persist.tile([P, T - jt * P], bf16, tag=f"et{jt}", name=f"et{jt}")
            for jt in range(NT)
        ]
        sums = persist.tile([P, NT, TCH], f32)
        rcol = persist.tile([P, NT, 1], f32)

        bq_sb = small.tile([P, FO], f32)
        bk_sb = small.tile([P, FO], f32)
        bv_sb = small.tile([P, E], bf16)
        msk_sb = small.tile([P, P], f32)

        # xt chunked so the first projection group can start after ~1MB;
        # weights ride a different DMA queue (gpsimd) than xt (sync).
        # First q-projection weight tile races ahead on the sync queue; xt
        # streams round-robin over three queues; remaining weights on gpsimd.
        w0 = wpool.tile([P, EO, P], bf16, tag="w", name="w0")
        nc.scalar.dma_start(w0[:], WQ[0])
        nc.scalar.dma_start(bq_sb[:], BQ[:])
        xt_dmas = [
            nc.sync.dma_start(xtc[ci][:], XT[:, :, ci * 512:(ci + 1) * 512])
            for ci in range(TCH)
        ]
        nc.scalar.dma_start(bk_sb[:], BK[:])
        nc.scalar.dma_start(bv_sb[:], BV[:])
        nc.scalar.dma_start(msk_sb[:], MSK[:])

        # ---- Phase 1a: qT, kT projections (feature-major) -------------------
        # qT[f, t] = sum_e Wq.T[e, f] * X.T[e, t]   (+ bq[f] per-partition)
        for W, bsb, dst in ((WQ, bq_sb, qt), (WK, bk_sb, kt)):
            for fo in range(FO):
                if W is WQ and fo == 0:
                    wti = w0
                else:
                    wti = wpool.tile([P, EO, P], bf16, tag="w")
                    nc.gpsimd.dma_start(wti[:], W[fo])
                for tci in range(TCH):
                    pt = ps.tile([P, 512], mybir.dt.float32, tag="ps")
                    for eo in range(EO):
                        nc.tensor.matmul(
                            pt[:],
                            wti[:, eo, :],
                            xtc[tci][:, eo, :],
                            start=(eo == 0),
                            stop=(eo == EO - 1),
                        )
                    # psum + per-partition bias -> bf16 SBUF (ScalarE)
                    nc.scalar.activation(
                        out=dst[:, fo, tci * 512:(tci + 1) * 512],
                        in_=pt[:],
                        func=Act.Identity,
                        bias=bsb[:, fo:fo + 1],
                    )

        # ---- Phase 1b: v projection (token-major) ---------------------------
        # v[t, f] = sum_e X.T[e, t] * Wv.T[e, f] + bv[f]
        wv_dma = nc.gpsimd.dma_start(wv[:], WV[:])
        # keep the 2MB wv transfer out of the startup-critical DMA window
        add_dep_helper(
            wv_dma.ins, xt_dmas[-1].ins, reason="defer wv load past xt stream"
        )
        for to in range(NT):
            for half in range(2):
                pt = ps.tile([P, 512], mybir.dt.float32, tag="ps")
                for eo in range(EO):
                    nc.tensor.matmul(
                        pt[:],
                        xtc[to // 4][:, eo, (to % 4) * P:(to % 4 + 1) * P],
                        wv[:, eo, half * 512:(half + 1) * 512],
                        start=(eo == 0),
                        stop=(eo == EO - 1),
                    )
                # psum + bv (free-axis broadcast row, pre-materialized) -> bf16
                nc.vector.tensor_add(
                    out=v[:, to, half * 512:(half + 1) * 512],
                    in0=pt[:],
                    in1=bv_sb[:, half * 512:(half + 1) * 512],
                )

        # ---- Phase 2+3 interleaved over key/query tiles ---------------------
        for jt in range(NT):
            n_i = T - jt * P          # live columns i >= jt*128
            nch = (n_i + 511) // 512
            for c in range(nch):
                i0 = jt * P + c * 512
                w = min(512, T - i0)
                pt = ps.tile([P, 512], mybir.dt.float32, tag="ps")
                for fo in range(FO):
                    nc.tensor.matmul(
                        pt[:, :w],
                        kt[:, fo, jt * P:(jt + 1) * P],
                        qt[:, fo, i0:i0 + w],
                        start=(fo == 0),
                        stop=(fo == FO - 1),
                    )
                if c == 0:
                    # causal mask on the diagonal 128x128 block (additive -1e30)
                    nc.vector.tensor_add(
                        out=pt[:, :P], in0=pt[:, :P], in1=msk_sb[:]
                    )
                nc.scalar.activation(
                    out=et[jt][:, c * 512:c * 512 + w],
                    in_=pt[:, :w],
                    func=Act.Exp,
                    scale=SCALE,
                    accum_out=sums[:, jt, c:c + 1],
                )
            # softmax denominator for this key tile; fold 1/colsum into v
            nc.vector.tensor_reduce(
                out=rcol[:, jt, :],
                in_=sums[:, jt, :nch],
                axis=mybir.AxisListType.X,
                op=mybir.AluOpType.add,
            )
            nc.vector.reciprocal(out=rcol[:, jt, :], in_=rcol[:, jt, :])
            nc.vector.tensor_scalar_mul(
                out=v[:, jt, :], in0=v[:, jt, :], scalar1=rcol[:, jt, :]
            )

            # Phase 3: out rows for query tile it == jt (needs et[0..jt], v'[0..jt])
            it = jt
            ob = outst.tile([P, E], mybir.dt.float32, tag="ob")
            for half in range(2):
                po = ps3.tile([P, 512], mybir.dt.float32, tag="ps3")
                for j2 in range(it + 1):
                    off = (it - j2) * P
                    nc.tensor.matmul(
                        po[:], et[j2][:, off:off + P],
                        v[:, j2, half * 512:(half + 1) * 512],
                        start=(j2 == 0), stop=(j2 == it),
                    )
                eng = nc.scalar if half == 0 else nc.vector
                if half == 0:
                    nc.scalar.copy(out=ob[:, 0:512], in_=po[:])
                    nc.scalar.dma_start(
                        OUT[it * P:(it + 1) * P, 0:512], ob[:, 0:512]
                    )
                else:
                    nc.vector.tensor_copy(out=ob[:, 512:1024], in_=po[:])
                    nc.sync.dma_start(
                        OUT[it * P:(it + 1) * P, 512:1024], ob[:, 512:1024]
                    )

    nc.compile()
    return nc


def _prep_inputs(X, Wq, bq, Wk, bk, Wv, bv):
    """Host-side reshape/cast into the layouts the device program expects."""
    X = np.asarray(X, dtype=np.float32)

    # XT[b, p, eo, t] = X[b, t, eo*128+p]
    xt4 = np.ascontiguousarray(
        X.transpose(0, 2, 1).reshape(B, EO, P, T).transpose(0, 2, 1, 3)
    ).astype(BF16)

    def wqk(Wm):
        # [fo, p, eo, c] = W.T[eo*128+p, fo*128+c]
        WmT = np.asarray(Wm, dtype=np.float32).T
        return np.ascontiguousarray(
            WmT.reshape(EO, P, FO, P).transpose(2, 1, 0, 3)
        ).astype(BF16)

    wq4 = wqk(Wq)
    wk4 = wqk(Wk)
    # WV: [p, eo, f] = Wv.T[eo*128+p, f]
    WvT = np.asarray(Wv, dtype=np.float32).T
    wv4 = np.ascontiguousarray(
        WvT.reshape(EO, P, E).transpose(1, 0, 2)
    ).astype(BF16)

    def b2(bm):
        return np.ascontiguousarray(
            np.asarray(bm, dtype=np.float32).reshape(FO, P).T
        )

    bq2 = b2(bq)
    bk2 = b2(bk)
    bvr = np.ascontiguousarray(
        np.broadcast_to(np.asarray(bv, dtype=np.float32), (P, E))
    ).astype(BF16)

    ii = np.arange(P)
    mask = np.where(ii[None, :] >= ii[:, None], 0.0, NEG).astype(np.float32)

    shared = {"WQ": wq4, "WK": wk4, "WV": wv4, "BQ": bq2, "BK": bk2,
              "BV": bvr, "MSK": mask}
    return [dict(shared, XT=np.ascontiguousarray(xt4[b])) for b in range(B)]


def run_sharded(inputs, trace=False, **kwargs):
    """Build (cached), run on 8 cores, gather. Returns (out, BassKernelResults)."""
    if "nc" not in _CACHE:
        _CACHE["nc"] = _build_nc()
    nc = _CACHE["nc"]
    in_maps = _prep_inputs(**inputs)
    res = run_bass_kernel_spmd(
        nc, in_maps, core_ids=list(range(B)), trace=trace, **kwargs
    )
    out = np.stack([np.asarray(r["OUT"], dtype=np.float32) for r in res.results])
    return out, res


def kernel(**inputs) -> np.ndarray:
    out, _ = run_sharded(inputs)
    return out
